# revision 26
# baseline (speedup 1.0000x reference)
"""BitLinear forward on 8 Trainium2 NeuronCores.

y = activation_quant(x) @ weight_quant(w).T

Strategy (column-parallel / tensor-parallel on out_features):
  - each core holds the full x and a 1/8 shard of w along out_features
  - weight scale (mean|w|) computed on-device with a 1-scalar AllReduce
  - activation quant: per-token int8 values, stored exactly in bf16
  - weight quant: ternary {-1,0,1}, stored exactly in bf16
  - matmul runs in bf16 (integer values -> exact fp32 accumulation in PSUM)
  - per-token output scale applied during PSUM drain
  - host concatenates the 8 output shards along out_features

The bf16 trick: quantized activations are integers in [-127,127] and
quantized weights are in {-1,0,1}; both are exactly representable in
bf16 and every partial sum is < 2^23, so the bf16 matmul with fp32
accumulation is bit-exact integer arithmetic at full PE rate.
"""

import numpy as np
from contextlib import ExitStack

import concourse.bass as bass
import concourse.tile as tile
from concourse import bacc, mybir
from concourse.bass import ts, ds
from concourse.bass_utils import run_bass_kernel_spmd

F32 = mybir.dt.float32
BF16 = mybir.dt.bfloat16
AX = mybir.AxisListType
OP = mybir.AluOpType
ACTF = mybir.ActivationFunctionType

# 1.5 * 2^23: adding then subtracting performs round-to-nearest-even at
# integer granularity in fp32 (matches jnp.round for |v| <= 2^21).
MAGIC = 12582912.0
EPS = 1e-5
P = 128


class _Builder:
    def __init__(self, nc, M, K, N, count_total, n_cores,
                 skip_quant=False, skip_matmul=False):
        self.nc = nc
        self.M, self.K, self.N = M, K, N
        self.count_total = count_total
        self.n_cores = n_cores
        self.skip_quant = skip_quant
        self.skip_matmul = skip_matmul
        self.KT = K // P
        self.MT = M // P
        self.NCH = 512
        self.NT = N // self.NCH
        self.MB = 256 if M % 256 == 0 else P
        self.MSUB = self.MB // P
        self.MBT = M // self.MB
        self.XH = K // 2
        self.WT = N // P

    def build(self, reps=1):
        nc = self.nc
        self.x = nc.dram_tensor("x", [self.M, self.K], F32,
                                kind="ExternalInput").ap()
        self.w = nc.dram_tensor("w", [self.N, self.K], F32,
                                kind="ExternalInput").ap()
        self.y = nc.dram_tensor("y", [self.M, self.N], F32,
                                kind="ExternalOutput").ap()

        with tile.TileContext(nc) as tc, ExitStack() as ctx:
            self.tc = tc
            dram = ctx.enter_context(
                tc.tile_pool(name="dram", bufs=1, space="DRAM"))
            self.xq_dram = dram.tile([self.M, self.K], BF16)
            self.wq_dram = dram.tile([self.N, self.K], BF16)
            self.cc_in = dram.tile([1, 1], F32)
            self.cc_out = dram.tile([1, 1], F32)

            const = ctx.enter_context(tc.tile_pool(name="const", bufs=1))
            self.onesf = const.tile([P, P], F32)
            nc.vector.memset(self.onesf[:], 1.0)
            self.alphas = const.tile([P, self.MT], F32)
            self.wsums = const.tile([P, self.WT], F32)
            self.s_w_eff = const.tile([P, 1], F32)
            self.s_inv = const.tile([P, 1], F32)
            self.const = const

            self.stat = ctx.enter_context(tc.tile_pool(name="stat", bufs=8))

            if self.skip_quant:
                # timing-ablation mode: seed tensors that quant would write
                nc.vector.memset(self.alphas[:], 1.0)
                seed = const.tile([P, 64], BF16)
                nc.vector.memset(seed[:], 1.0)
                nc.gpsimd.dma_start(self.xq_dram[0:P, 0:64], seed[:])
                nc.gpsimd.dma_start(self.wq_dram[0:P, 0:64], seed[:])

            for _ in range(reps):
                self.emit_once(ctx)
        return nc

    def emit_once(self, ctx):
        if not self.skip_quant:
            self.emit_wscale_and_quant()
        with ExitStack() as mctx:
            if not self.skip_matmul:
                self.emit_main(mctx)
            elif not self.skip_quant:
                self.emit_xquant_standalone(mctx)

    # ---------------- weight scale + ternary quant ----------------------
    def emit_wscale_and_quant(self):
        nc, tc = self.nc, self.tc
        with ExitStack() as wctx:
            wp = wctx.enter_context(tc.tile_pool(name="w_in", bufs=2))
            wq_st = wctx.enter_context(tc.tile_pool(name="wq_st", bufs=2))
            self.pssmall = wctx.enter_context(
                tc.tile_pool(name="pssmall", bufs=1, space="PSUM"))

            for t in range(self.WT):
                wt = wp.tile([P, self.K], F32, tag="w", name="wt")
                nc.sync.dma_start(wt[:], self.w[ts(t, P), :])
                nc.vector.tensor_reduce(
                    self.wsums[:, ds(t, 1)], wt[:], axis=AX.X, op=OP.add,
                    apply_absolute_value=True)

            wpart = self.stat.tile([P, 1], F32, tag="wpart", name="wpart")
            nc.vector.tensor_reduce(
                wpart[:], self.wsums[:], axis=AX.X, op=OP.add)
            ps1 = self.pssmall.tile([1, 1], F32, tag="small", name="ps1")
            # ones[128,1].T @ wpart[128,1] -> total |w| sum on this core
            nc.tensor.matmul(
                ps1[:], self.onesf[:, 0:1], wpart[:], start=True, stop=True)
            ccs = self.stat.tile([1, 1], F32, tag="ccs", name="ccs")
            nc.vector.tensor_copy(ccs[:], ps1[:])
            nc.gpsimd.dma_start(self.cc_in[:], ccs[:])
            if self.n_cores > 1:
                nc.gpsimd.collective_compute(
                    "AllReduce", OP.add,
                    replica_groups=[list(range(self.n_cores))],
                    ins=[self.cc_in.opt()], outs=[self.cc_out.opt()])
            else:
                nc.gpsimd.dma_start(self.cc_out[:], self.cc_in[:])
            ccb = self.stat.tile([1, 1], F32, tag="ccb", name="ccb")
            nc.sync.dma_start(ccb[:], self.cc_out[:])
            ps2 = self.pssmall.tile([P, 1], F32, tag="small", name="ps2")
            # ones[1,128].T @ val[1,1] -> broadcast scalar to 128 partitions
            nc.tensor.matmul(
                ps2[:], self.onesf[0:1, :], ccb[:], start=True, stop=True)
            # mean = sum/count; count is a power of two so multiply is exact
            assert self.count_total & (self.count_total - 1) == 0
            s_w = self.stat.tile([P, 1], F32, tag="s_w", name="s_w")
            nc.vector.tensor_scalar(
                s_w[:], ps2[:], 1.0 / float(self.count_total), EPS,
                OP.mult, OP.max)
            nc.vector.reciprocal(self.s_inv[:], s_w[:])
            nc.vector.reciprocal(self.s_w_eff[:], self.s_inv[:])

            # wq = clip(round(w * s_inv), -1, 1), stored bf16
            for t in range(self.WT):
                wt = wp.tile([P, self.K], F32, tag="w", name="wt")
                nc.sync.dma_start(wt[:], self.w[ts(t, P), :])
                nc.scalar.activation(
                    wt[:], wt[:], ACTF.Copy, scale=self.s_inv[:, 0:1])
                nc.vector.tensor_scalar(
                    wt[:], wt[:], MAGIC, -MAGIC, OP.add, OP.add)
                wqs = wq_st.tile([P, self.K], BF16, tag="wq", name="wqs")
                nc.vector.tensor_scalar(
                    wqs[:], wt[:], 1.0, -1.0, OP.min, OP.max)
                nc.gpsimd.dma_start(self.wq_dram[ts(t, P), :], wqs[:])

    # ---------------- activation quant for one 128-token tile ------------
    def quant_x(self, mt, xpool, xqpool):
        nc = self.nc
        stat = self.stat
        xs = []
        for h in range(2):
            xt = xpool.tile([P, self.XH], F32, tag="x", name="xt")
            nc.sync.dma_start(
                xt[:], self.x[ts(mt, P), ds(h * self.XH, self.XH)])
            xs.append(xt)
        am0 = stat.tile([P, 1], F32, tag="am0", name="am0")
        am1 = stat.tile([P, 1], F32, tag="am1", name="am1")
        nc.vector.tensor_reduce(
            am0[:], xs[0][:], axis=AX.X, op=OP.max, apply_absolute_value=True)
        nc.vector.tensor_reduce(
            am1[:], xs[1][:], axis=AX.X, op=OP.max, apply_absolute_value=True)
        amc = stat.tile([P, 1], F32, tag="amc", name="amc")
        nc.vector.tensor_tensor(amc[:], am0[:], am1[:], OP.max)
        ame = stat.tile([P, 1], F32, tag="ame", name="ame")
        nc.vector.tensor_scalar(ame[:], amc[:], EPS, None, OP.max)
        amr = stat.tile([P, 1], F32, tag="amr", name="amr")
        nc.vector.reciprocal(amr[:], ame[:])
        scale = stat.tile([P, 1], F32, tag="scale", name="scale")
        nc.vector.tensor_scalar(scale[:], amr[:], 127.0, None, OP.mult)
        inv = stat.tile([P, 1], F32, tag="inv", name="inv")
        nc.vector.reciprocal(inv[:], scale[:])
        nc.vector.tensor_tensor(
            self.alphas[:, ds(mt, 1)], inv[:], self.s_w_eff[:], OP.mult)
        for h in range(2):
            nc.scalar.activation(
                xs[h][:], xs[h][:], ACTF.Copy, scale=scale[:, 0:1])
            xq = xqpool.tile([P, self.XH], BF16, tag="xq", name="xq")
            nc.vector.tensor_scalar(
                xq[:], xs[h][:], MAGIC, -MAGIC, OP.add, OP.add)
            nc.gpsimd.dma_start(
                self.xq_dram[ts(mt, P), ds(h * self.XH, self.XH)], xq[:])

    # ---------------- transposed weights + x quant + matmuls -------------
    def emit_main(self, ctx):
        nc, tc = self.nc, self.tc
        wqtpool = ctx.enter_context(tc.tile_pool(name="wqt", bufs=1))
        wqt = wqtpool.tile([P, self.KT, self.N], BF16)
        for k in range(self.KT):
            nc.sync.dma_start_transpose(
                wqt[:, k, :], self.wq_dram[:, ds(k * P, P)])

        xpool = ctx.enter_context(tc.tile_pool(name="x_in", bufs=3))
        xqpool = ctx.enter_context(tc.tile_pool(name="xq_st", bufs=3))
        xqtpool = ctx.enter_context(tc.tile_pool(name="xqt", bufs=2))
        pspool = ctx.enter_context(tc.tile_pool(name="ps", bufs=2, space="PSUM"))
        outpool = ctx.enter_context(tc.tile_pool(name="out", bufs=4))

        LOOK = 2  # quant runs this many blocks ahead of the matmul stream

        def quant_block(b):
            if not self.skip_quant and b < self.MBT:
                for ms in range(self.MSUB):
                    self.quant_x(b * self.MSUB + ms, xpool, xqpool)

        for j in range(LOOK):
            quant_block(j)

        for mb in range(self.MBT):
            quant_block(mb + LOOK)
            xqts = []
            for k in range(self.KT):
                xt = xqtpool.tile([P, self.MB], BF16, tag=f"k{k}", name="xqt")
                nc.sync.dma_start_transpose(
                    xt[:], self.xq_dram[ds(mb * self.MB, self.MB), ds(k * P, P)])
                xqts.append(xt)
            for ms in range(self.MSUB):
                mt = mb * self.MSUB + ms
                psums = [
                    pspool.tile([P, self.NCH], F32, tag=f"ps{nn}", name=f"ps{nn}")
                    for nn in range(self.NT)
                ]
                for k in range(self.KT):
                    lhs = xqts[k][:, ds(ms * P, P)]
                    first, last = (k == 0), (k == self.KT - 1)
                    for nn in range(self.NT):
                        nc.tensor.matmul(
                            psums[nn][:], lhs,
                            wqt[:, k, ds(nn * self.NCH, self.NCH)],
                            start=first, stop=last)
                for nn in range(self.NT):
                    ot = outpool.tile([P, self.NCH], F32, tag="o", name="ot")
                    nc.scalar.activation(
                        ot[:], psums[nn][:], ACTF.Copy,
                        scale=self.alphas[:, ds(mt, 1)])
                    nc.gpsimd.dma_start(
                        self.y[ts(mt, P), ds(nn * self.NCH, self.NCH)], ot[:])

    def emit_xquant_standalone(self, ctx):
        tc = self.tc
        xpool = ctx.enter_context(tc.tile_pool(name="x_in", bufs=3))
        xqpool = ctx.enter_context(tc.tile_pool(name="xq_st", bufs=3))
        for mt in range(self.MT):
            self.quant_x(mt, xpool, xqpool)


def build_bitlinear(nc, M, K, N, count_total, n_cores, reps=1,
                    skip_quant=False, skip_matmul=False):
    return _Builder(nc, M, K, N, count_total, n_cores,
                    skip_quant=skip_quant, skip_matmul=skip_matmul).build(reps)


# ----------------------------------------------------------------------------
# Host-side entry point
# ----------------------------------------------------------------------------

_FULL = dict(B=4, S=2048, K=4096, N_TOTAL=16384, N_CORES=8)
_CACHE = {}


def _make_nc(reps=1, skip_quant=False, skip_matmul=False):
    cfg = _FULL
    M = cfg["B"] * cfg["S"]
    n_shard = cfg["N_TOTAL"] // cfg["N_CORES"]
    nc = bacc.Bacc(
        "TRN2",
        target_bir_lowering=False,
        debug=False,
        num_devices=cfg["N_CORES"],
    )
    build_bitlinear(
        nc, M=M, K=cfg["K"], N=n_shard,
        count_total=cfg["N_TOTAL"] * cfg["K"],
        n_cores=cfg["N_CORES"],
        reps=reps, skip_quant=skip_quant, skip_matmul=skip_matmul,
    )
    nc.compile()
    from concourse.bass_interp import get_hw_module
    nc.m = get_hw_module(nc.m)
    return nc


def _get_compiled():
    if "nc" not in _CACHE:
        _CACHE["nc"] = _make_nc()
    return _CACHE["nc"]


def kernel(x: np.ndarray, weight: np.ndarray, _trace: bool = False):
    cfg = _FULL
    M = cfg["B"] * cfg["S"]
    n_shard = cfg["N_TOTAL"] // cfg["N_CORES"]
    nc = _get_compiled()

    x2 = np.ascontiguousarray(np.asarray(x, dtype=np.float32).reshape(M, cfg["K"]))
    wf = np.asarray(weight, dtype=np.float32)
    in_maps = [
        {"x": x2, "w": np.ascontiguousarray(wf[i * n_shard:(i + 1) * n_shard])}
        for i in range(cfg["N_CORES"])
    ]
    res = run_bass_kernel_spmd(
        nc, in_maps, list(range(cfg["N_CORES"])), trace=_trace)
    _CACHE["last_result"] = res
    yfull = np.concatenate(
        [res.results[i]["y"] for i in range(cfg["N_CORES"])], axis=1)
    return yfull.reshape(cfg["B"], cfg["S"], cfg["N_TOTAL"])


# revision 28
# speedup vs baseline: 1.6614x; 1.6614x over previous
"""BitLinear forward on 8 Trainium2 NeuronCores.

y = activation_quant(x) @ weight_quant(w).T

Strategy (column-parallel / tensor-parallel on out_features):
  - each core holds the full x and a 1/8 shard of w along out_features
  - weight scale (mean|w|) computed on-device with a 1-scalar AllReduce
  - activation quant: per-token int8 values, stored exactly in bf16
  - weight quant: ternary {-1,0,1}, stored exactly in bf16
  - matmul runs in bf16 (integer values -> exact fp32 accumulation in PSUM)
  - per-token output scale applied during PSUM drain
  - host concatenates the 8 output shards along out_features

The bf16 trick: quantized activations are integers in [-127,127] and
quantized weights are in {-1,0,1}; both are exactly representable in
bf16 and every partial sum is < 2^23, so the bf16 matmul with fp32
accumulation is bit-exact integer arithmetic at full PE rate.
"""

import numpy as np
from contextlib import ExitStack

import concourse.bass as bass
import concourse.tile as tile
from concourse import bacc, mybir
from concourse.bass import ts, ds
from concourse.bass_utils import run_bass_kernel_spmd

F32 = mybir.dt.float32
BF16 = mybir.dt.bfloat16
AX = mybir.AxisListType
OP = mybir.AluOpType
ACTF = mybir.ActivationFunctionType

# 1.5 * 2^23: adding then subtracting performs round-to-nearest-even at
# integer granularity in fp32 (matches jnp.round for |v| <= 2^21).
MAGIC = 12582912.0
EPS = 1e-5
P = 128


class _Builder:
    def __init__(self, nc, M, K, N, count_total, n_cores,
                 skip_quant=False, skip_matmul=False):
        self.nc = nc
        self.M, self.K, self.N = M, K, N
        self.count_total = count_total
        self.n_cores = n_cores
        self.skip_quant = skip_quant
        self.skip_matmul = skip_matmul
        self.KT = K // P
        self.MT = M // P
        self.NCH = 512
        self.NT = N // self.NCH
        self.MB = 256 if M % 256 == 0 else P
        self.MSUB = self.MB // P
        self.MBT = M // self.MB
        self.XH = K // 2
        self.WT = N // P

    def build(self, reps=1):
        nc = self.nc
        self.x = nc.dram_tensor("x", [self.M, self.K], F32,
                                kind="ExternalInput").ap()
        self.w = nc.dram_tensor("w", [self.N, self.K], F32,
                                kind="ExternalInput").ap()
        self.y = nc.dram_tensor("y", [self.M, self.N], F32,
                                kind="ExternalOutput").ap()

        with tile.TileContext(nc) as tc, ExitStack() as ctx:
            self.tc = tc
            dram = ctx.enter_context(
                tc.tile_pool(name="dram", bufs=1, space="DRAM"))
            self.xq_dram = dram.tile([self.M, self.K], BF16)
            self.wq_dram = dram.tile([self.N, self.K], BF16)
            self.cc_in = dram.tile([1, 1], F32)
            self.cc_out = dram.tile([1, 1], F32)

            const = ctx.enter_context(tc.tile_pool(name="const", bufs=1))
            self.onesf = const.tile([P, P], F32)
            nc.vector.memset(self.onesf[:], 1.0)
            self.alphas = const.tile([P, self.MT], F32)
            self.wsums = const.tile([P, self.WT], F32)
            self.s_w_eff = const.tile([P, 1], F32)
            self.s_inv = const.tile([P, 1], F32)
            self.const = const

            self.stat = ctx.enter_context(tc.tile_pool(name="stat", bufs=8))

            if self.skip_quant:
                # timing-ablation mode: seed tensors that quant would write
                nc.vector.memset(self.alphas[:], 1.0)
                seed = const.tile([P, 64], BF16)
                nc.vector.memset(seed[:], 1.0)
                nc.gpsimd.dma_start(self.xq_dram[0:P, 0:64], seed[:])
                nc.gpsimd.dma_start(self.wq_dram[0:P, 0:64], seed[:])

            for _ in range(reps):
                self.emit_once(ctx)
        return nc

    def emit_once(self, ctx):
        if not self.skip_quant:
            self.emit_wscale_and_quant()
        with ExitStack() as mctx:
            if not self.skip_matmul:
                self.emit_main(mctx)
            elif not self.skip_quant:
                self.emit_xquant_standalone(mctx)

    # ---------------- weight scale + ternary quant ----------------------
    def emit_wscale_and_quant(self):
        nc, tc = self.nc, self.tc
        with ExitStack() as wctx:
            wp = wctx.enter_context(tc.tile_pool(name="w_in", bufs=2))
            wq_st = wctx.enter_context(tc.tile_pool(name="wq_st", bufs=2))
            self.pssmall = wctx.enter_context(
                tc.tile_pool(name="pssmall", bufs=1, space="PSUM"))

            for t in range(self.WT):
                wt = wp.tile([P, self.K], F32, tag="w", name="wt")
                nc.sync.dma_start(wt[:], self.w[ts(t, P), :])
                nc.vector.tensor_reduce(
                    self.wsums[:, ds(t, 1)], wt[:], axis=AX.X, op=OP.add,
                    apply_absolute_value=True)

            wpart = self.stat.tile([P, 1], F32, tag="wpart", name="wpart")
            nc.vector.tensor_reduce(
                wpart[:], self.wsums[:], axis=AX.X, op=OP.add)
            ps1 = self.pssmall.tile([1, 1], F32, tag="small", name="ps1")
            # ones[128,1].T @ wpart[128,1] -> total |w| sum on this core
            nc.tensor.matmul(
                ps1[:], self.onesf[:, 0:1], wpart[:], start=True, stop=True)
            ccs = self.stat.tile([1, 1], F32, tag="ccs", name="ccs")
            nc.vector.tensor_copy(ccs[:], ps1[:])
            nc.gpsimd.dma_start(self.cc_in[:], ccs[:])
            if self.n_cores > 1:
                nc.gpsimd.collective_compute(
                    "AllReduce", OP.add,
                    replica_groups=[list(range(self.n_cores))],
                    ins=[self.cc_in.opt()], outs=[self.cc_out.opt()])
            else:
                nc.gpsimd.dma_start(self.cc_out[:], self.cc_in[:])
            ccb = self.stat.tile([1, 1], F32, tag="ccb", name="ccb")
            nc.sync.dma_start(ccb[:], self.cc_out[:])
            ps2 = self.pssmall.tile([P, 1], F32, tag="small", name="ps2")
            # ones[1,128].T @ val[1,1] -> broadcast scalar to 128 partitions
            nc.tensor.matmul(
                ps2[:], self.onesf[0:1, :], ccb[:], start=True, stop=True)
            # mean = sum/count; count is a power of two so multiply is exact
            assert self.count_total & (self.count_total - 1) == 0
            s_w = self.stat.tile([P, 1], F32, tag="s_w", name="s_w")
            nc.vector.tensor_scalar(
                s_w[:], ps2[:], 1.0 / float(self.count_total), EPS,
                OP.mult, OP.max)
            nc.vector.reciprocal(self.s_inv[:], s_w[:])
            nc.vector.reciprocal(self.s_w_eff[:], self.s_inv[:])

            # wq = clip(round(w * s_inv), -1, 1), stored bf16
            for t in range(self.WT):
                wt = wp.tile([P, self.K], F32, tag="w", name="wt")
                nc.sync.dma_start(wt[:], self.w[ts(t, P), :])
                nc.scalar.activation(
                    wt[:], wt[:], ACTF.Copy, scale=self.s_inv[:, 0:1])
                nc.vector.tensor_scalar(
                    wt[:], wt[:], MAGIC, -MAGIC, OP.add, OP.add)
                wqs = wq_st.tile([P, self.K], BF16, tag="wq", name="wqs")
                nc.vector.tensor_scalar(
                    wqs[:], wt[:], 1.0, -1.0, OP.min, OP.max)
                nc.gpsimd.dma_start(self.wq_dram[ts(t, P), :], wqs[:])

    # ---------------- activation quant for one 128-token tile ------------
    def quant_x(self, mt, xpool, xqpool):
        nc = self.nc
        stat = self.stat
        xs = []
        for h in range(2):
            xt = xpool.tile([P, self.XH], F32, tag="x", name="xt")
            nc.sync.dma_start(
                xt[:], self.x[ts(mt, P), ds(h * self.XH, self.XH)])
            xs.append(xt)
        am0 = stat.tile([P, 1], F32, tag="am0", name="am0")
        am1 = stat.tile([P, 1], F32, tag="am1", name="am1")
        nc.vector.tensor_reduce(
            am0[:], xs[0][:], axis=AX.X, op=OP.max, apply_absolute_value=True)
        nc.vector.tensor_reduce(
            am1[:], xs[1][:], axis=AX.X, op=OP.max, apply_absolute_value=True)
        amc = stat.tile([P, 1], F32, tag="amc", name="amc")
        nc.vector.tensor_tensor(amc[:], am0[:], am1[:], OP.max)
        ame = stat.tile([P, 1], F32, tag="ame", name="ame")
        nc.vector.tensor_scalar(ame[:], amc[:], EPS, None, OP.max)
        amr = stat.tile([P, 1], F32, tag="amr", name="amr")
        nc.vector.reciprocal(amr[:], ame[:])
        scale = stat.tile([P, 1], F32, tag="scale", name="scale")
        nc.vector.tensor_scalar(scale[:], amr[:], 127.0, None, OP.mult)
        inv = stat.tile([P, 1], F32, tag="inv", name="inv")
        nc.vector.reciprocal(inv[:], scale[:])
        nc.vector.tensor_tensor(
            self.alphas[:, ds(mt, 1)], inv[:], self.s_w_eff[:], OP.mult)
        for h in range(2):
            nc.scalar.activation(
                xs[h][:], xs[h][:], ACTF.Copy, scale=scale[:, 0:1])
            xq = xqpool.tile([P, self.XH], BF16, tag="xq", name="xq")
            nc.vector.tensor_scalar(
                xq[:], xs[h][:], MAGIC, -MAGIC, OP.add, OP.add)
            nc.gpsimd.dma_start(
                self.xq_dram[ts(mt, P), ds(h * self.XH, self.XH)], xq[:])

    # ---------------- transposed weights + x quant + matmuls -------------
    def emit_main(self, ctx):
        nc, tc = self.nc, self.tc
        wqtpool = ctx.enter_context(tc.tile_pool(name="wqt", bufs=1))
        wqt = wqtpool.tile([P, self.KT, self.N], BF16)
        for k in range(self.KT):
            nc.sync.dma_start_transpose(
                wqt[:, k, :], self.wq_dram[:, ds(k * P, P)])

        xpool = ctx.enter_context(tc.tile_pool(name="x_in", bufs=3))
        xqpool = ctx.enter_context(tc.tile_pool(name="xq_st", bufs=3))
        xqtpool = ctx.enter_context(tc.tile_pool(name="xqt", bufs=2))
        pspool = ctx.enter_context(tc.tile_pool(name="ps", bufs=2, space="PSUM"))
        outpool = ctx.enter_context(tc.tile_pool(name="out", bufs=4))

        LOOK = 2  # quant runs this many blocks ahead of the matmul stream

        def quant_block(b):
            if not self.skip_quant and b < self.MBT:
                for ms in range(self.MSUB):
                    self.quant_x(b * self.MSUB + ms, xpool, xqpool)

        for j in range(LOOK):
            quant_block(j)

        for mb in range(self.MBT):
            quant_block(mb + LOOK)
            xqts = []
            for k in range(self.KT):
                xt = xqtpool.tile([P, self.MB], BF16, tag=f"k{k}", name="xqt")
                nc.sync.dma_start_transpose(
                    xt[:], self.xq_dram[ds(mb * self.MB, self.MB), ds(k * P, P)])
                xqts.append(xt)
            for ms in range(self.MSUB):
                mt = mb * self.MSUB + ms
                psums = [
                    pspool.tile([P, self.NCH], F32, tag=f"ps{nn}", name=f"ps{nn}")
                    for nn in range(self.NT)
                ]
                for k in range(self.KT):
                    lhs = xqts[k][:, ds(ms * P, P)]
                    first, last = (k == 0), (k == self.KT - 1)
                    for nn in range(self.NT):
                        nc.tensor.matmul(
                            psums[nn][:], lhs,
                            wqt[:, k, ds(nn * self.NCH, self.NCH)],
                            start=first, stop=last)
                for nn in range(self.NT):
                    ot = outpool.tile([P, self.NCH], F32, tag="o", name="ot")
                    nc.scalar.activation(
                        ot[:], psums[nn][:], ACTF.Copy,
                        scale=self.alphas[:, ds(mt, 1)])
                    nc.gpsimd.dma_start(
                        self.y[ts(mt, P), ds(nn * self.NCH, self.NCH)], ot[:])

    def emit_xquant_standalone(self, ctx):
        tc = self.tc
        xpool = ctx.enter_context(tc.tile_pool(name="x_in", bufs=3))
        xqpool = ctx.enter_context(tc.tile_pool(name="xq_st", bufs=3))
        for mt in range(self.MT):
            self.quant_x(mt, xpool, xqpool)


def build_bitlinear(nc, M, K, N, count_total, n_cores, reps=1,
                    skip_quant=False, skip_matmul=False):
    return _Builder(nc, M, K, N, count_total, n_cores,
                    skip_quant=skip_quant, skip_matmul=skip_matmul).build(reps)


def dedupe_ldweights(nc):
    """Drop InstLdweights that reload the exact weights already resident in
    the PE array. The tile pipeline splits every bf16 matmul into
    LDWEIGHTS+MATMUL(ldweights=False); consecutive matmuls sharing one
    stationary operand (the nn loop) therefore reload it redundantly.
    The stationary operand persists across non-self-loading matmuls, so a
    sync-free LDW identical to the previous one is a no-op. Run after
    nc.compile() so all semaphore passes have finalized sync_info."""
    removed = 0
    for fn in nc.m.functions:
        for blk in fn.blocks:
            last_sig = None
            keep = []
            for inst in blk.instructions:
                if isinstance(inst, mybir.InstLdweights):
                    a = inst.ins[0]
                    sig = (getattr(a, "memref", None), a.offset, str(a.ap),
                           str(a.dtype), str(inst.perf_mode),
                           str(inst.tile_position), str(inst.is_transpose))
                    si = inst.sync_info
                    clean = si is None or (not si.on_wait and not si.on_update)
                    if sig == last_sig and clean and sig[0] is not None:
                        removed += 1
                        continue
                    last_sig = sig
                elif isinstance(inst, mybir.InstMatmult):
                    if inst.ldweights is not False:
                        last_sig = None  # self-loading matmul clobbers weights
                elif getattr(inst, "engine", None) == mybir.EngineType.PE:
                    if inst.is_executable():
                        last_sig = None  # unknown PE instruction: be safe
                keep.append(inst)
            if len(keep) != len(blk.instructions):
                blk.instructions = keep
    return removed


# ----------------------------------------------------------------------------
# Host-side entry point
# ----------------------------------------------------------------------------

_FULL = dict(B=4, S=2048, K=4096, N_TOTAL=16384, N_CORES=8)
_CACHE = {}


def _make_nc(reps=1, skip_quant=False, skip_matmul=False):
    cfg = _FULL
    M = cfg["B"] * cfg["S"]
    n_shard = cfg["N_TOTAL"] // cfg["N_CORES"]
    nc = bacc.Bacc(
        "TRN2",
        target_bir_lowering=False,
        debug=False,
        num_devices=cfg["N_CORES"],
    )
    build_bitlinear(
        nc, M=M, K=cfg["K"], N=n_shard,
        count_total=cfg["N_TOTAL"] * cfg["K"],
        n_cores=cfg["N_CORES"],
        reps=reps, skip_quant=skip_quant, skip_matmul=skip_matmul,
    )
    nc.compile()
    dedupe_ldweights(nc)
    from concourse.bass_interp import get_hw_module
    nc.m = get_hw_module(nc.m)
    return nc


def _get_compiled():
    if "nc" not in _CACHE:
        _CACHE["nc"] = _make_nc()
    return _CACHE["nc"]


def kernel(x: np.ndarray, weight: np.ndarray, _trace: bool = False):
    cfg = _FULL
    M = cfg["B"] * cfg["S"]
    n_shard = cfg["N_TOTAL"] // cfg["N_CORES"]
    nc = _get_compiled()

    x2 = np.ascontiguousarray(np.asarray(x, dtype=np.float32).reshape(M, cfg["K"]))
    wf = np.asarray(weight, dtype=np.float32)
    in_maps = [
        {"x": x2, "w": np.ascontiguousarray(wf[i * n_shard:(i + 1) * n_shard])}
        for i in range(cfg["N_CORES"])
    ]
    res = run_bass_kernel_spmd(
        nc, in_maps, list(range(cfg["N_CORES"])), trace=_trace)
    _CACHE["last_result"] = res
    yfull = np.concatenate(
        [res.results[i]["y"] for i in range(cfg["N_CORES"])], axis=1)
    return yfull.reshape(cfg["B"], cfg["S"], cfg["N_TOTAL"])
